# revision 19
# baseline (speedup 1.0000x reference)
"""Trainium2 Bass kernel for nn_DiffusionLayer (N=8192, D=128), 8-core SPMD.

Computation:
    t = relu(Z @ W1 + b1) @ W2 + b2      # [N, D]  (the MLP "transform")
    S = softmax(t @ t.T, axis=1)         # [N, N]
    out = Z + TAU * (S @ Z - Z)

Sharding: output rows split across 8 NeuronCores; each core computes its
1024-row S block against a replicated transform t (computed on host, 0.8%
of FLOPs) — flash-attention-style sequence parallelism.

Device pipeline per core (v2 — dual-engine exp):
  - t^T DMA'd straight into SBUF as fp32r (no cast pass), priority-ordered
    so the first sim matmuls start ~2us in; PE warmed up with dummy
    matmuls during the DMA window so real matmuls run at full DVFS clock.
  - sim^T tiles in groups of 4 j-tiles x 256 i-chunk via fp32r matmuls
    (ap>=256 -> 1 cyc/row), triple-buffered 2-bank PSUM groups.
  - exp is split across TWO engines (the Act engine was the baseline
    bottleneck at ~67us): Act computes bf16 exp(sim - C) for 10/16 groups;
    DVE computes the rest via a Schraudolph-style bitwise exp: one fused
    tensor_scalar (sim*s + b) with saturating round-to-nearest fp32->uint16
    cast whose result IS the bf16 bit pattern of exp(sim-C) (validated on
    hw: negatives saturate to 0 -> +0.0, rel err ~1.3e-3 vs 2e-2 budget).
  - PV: E-slice^T @ [Zh | 1] in bf16 lagged LAG groups behind exp,
    interleaved with sim on the PE so the tail is only ~LAG groups.
  - normalize: DVE reciprocal -> Act Copy-with-scale (u = pv * TAU/den)
    -> DVE residual stt -> DMA out, per chunk.
"""

import sys

sys.path.insert(0, "/opt/trn_rl_repo")

import numpy as np
import ml_dtypes
import orjson
from contextlib import ExitStack

import concourse.bass as bass
import concourse.tile as tile
from concourse import mybir
from concourse.bass_utils import run_bass_kernel_spmd

F32 = mybir.dt.float32
F32R = mybir.dt.float32r
BF16 = mybir.dt.bfloat16
U16 = mybir.dt.uint16
BF = ml_dtypes.bfloat16

N, D = 8192, 128
NCORES = 8
BLK = N // NCORES  # 1024 rows per core
NT = N // 128  # 64 j-tiles
NBT = BLK // 128  # 8 row tiles of the block
TAU = 0.1

CH = 256  # i-chunk width
NCH = BLK // CH  # 4 chunks per core
GJ = 4  # j-tiles per sim PSUM group (2 banks)
NG = NT // GJ  # 16 groups per chunk
LAG = 2  # PV trails exp by LAG groups
DVE_GROUPS = {1, 3, 5, 7, 9, 11, 13, 15}  # groups per chunk computed on DVE
N_WARMUP = 10  # PE DVFS warmup matmuls

S_SCHR = float(2.0**7 / np.log(2.0))  # 184.6650...
C_CORR = 5.5  # Schraudolph bias calibration (RTNE cast)

# ---------------------------------------------------------------------------
# BIR post-pass: the walrus build in this image encodes at most one sync wait
# per instruction; Tile emits several on some instructions. Split excess
# waits onto preceding same-engine NoOp carriers.
_MAX_WAITS = 1


def _split_multiwaits(m: dict) -> bool:
    changed = False
    counter = [0]

    def fresh_name():
        counter[0] += 1
        return f"I-waitsplit-{counter[0]}"

    for fn in m.get("functions", []):
        for bb in fn.get("blocks", []):
            out = []
            for inst in bb.get("instructions", []):
                si = inst.get("sync_info") or {}
                waits = si.get("on_wait") or []
                if len(waits) > _MAX_WAITS:
                    changed = True
                    head, tail = waits[:-_MAX_WAITS], waits[-_MAX_WAITS:]
                    for i in range(0, len(head), _MAX_WAITS):
                        out.append(
                            {
                                "debug": inst.get("debug", 0),
                                "engine": inst["engine"],
                                "ins": [],
                                "name": fresh_name(),
                                "opcode": "NoOp",
                                "outs": [],
                                "sync_info": {
                                    "on_update": [],
                                    "on_wait": head[i : i + _MAX_WAITS],
                                },
                            }
                        )
                    si["on_wait"] = tail
                out.append(inst)
            bb["instructions"] = out
    return changed


def _patch_nc(nc):
    orig = nc.to_json_bytes

    def to_json_bytes_fixed():
        m = orjson.loads(orig())
        if _split_multiwaits(m):
            return orjson.dumps(m)
        return orig()

    nc.to_json_bytes = to_json_bytes_fixed
    return nc


# ---------------------------------------------------------------------------


def _build_nc(c_shift: float):
    nc = bass.Bass("TRN2", debug=False, num_devices=NCORES)

    Ttd = nc.dram_tensor("Tt", [D, N], BF16, kind="ExternalInput").ap()
    Tbtd = nc.dram_tensor("Tbt", [D, BLK], BF16, kind="ExternalInput").ap()
    Zad = nc.dram_tensor("Za", [N, D + 1], BF16, kind="ExternalInput").ap()
    Zbd = nc.dram_tensor("Zb", [BLK, D], F32, kind="ExternalInput").ap()
    Od = nc.dram_tensor("O", [BLK, D], F32, kind="ExternalOutput").ap()

    Zar = Zad.rearrange("(t p) e -> p t e", p=128)  # [128, 64, 129]
    Zbr = Zbd.rearrange("(t p) d -> p t d", p=128)  # [128, 8, 128]
    Or = Od.rearrange("(t p) d -> p t d", p=128)

    b_schr = float(127 * 128 - C_CORR - c_shift * S_SCHR)

    with tile.TileContext(nc) as tc, ExitStack() as ctx:
        const = ctx.enter_context(tc.tile_pool(name="const", bufs=1))
        sb = ctx.enter_context(tc.tile_pool(name="sb", bufs=1))
        ebig = ctx.enter_context(tc.tile_pool(name="ebig", bufs=2))
        # PSUM budget (8 banks): simps 2 bufs x 2 banks + pvps 2 bufs x 2
        # banks. Each PV row-slice accumulator gets its OWN bank (512-elem
        # stride pad): interleaved accumulation groups sharing a bank
        # corrupt each other (start zeroes bank-wide).
        simps = ctx.enter_context(tc.tile_pool(name="simps", bufs=2, space="PSUM"))
        pvps = ctx.enter_context(tc.tile_pool(name="pvps", bufs=2, space="PSUM"))

        # ---- constants + Act table preload
        dummy = const.tile([128, 1], F32)
        nc.vector.memset(dummy[:], 0.0)
        dummy2 = const.tile([128, 1], F32)
        nc.scalar.activation(dummy2[:], dummy[:], mybir.ActivationFunctionType.Exp)
        cbias = const.tile([128, 1], F32)  # per-partition exp bias = -C
        nc.vector.memset(cbias[:], -c_shift)
        wl = const.tile([128, 128], BF16)
        nc.vector.memset(wl[:].bitcast(U16), 0)
        wr = const.tile([128, 2 * (D + 1)], BF16)
        nc.vector.memset(wr[:].bitcast(U16), 0)

        # ---- persistent SBUF tensors
        t_sb = sb.tile([128, N], BF16)  # t^T [d, N] (bf16: halves the stream)
        tb_sb = sb.tile([128, BLK], BF16)  # t_blk^T (this core's columns)
        zaug = sb.tile([128, NT, D + 1], BF16)  # [Zh | 1] row tiles
        zbn = sb.tile([128, NBT, 128], F32)  # Z block (residual)
        u_sb = sb.tile([128, NBT, 128], F32)  # TAU/den-scaled PV rows
        o_sb = sb.tile([128, NBT, 128], F32)
        rec = sb.tile([128, NBT, 1], F32)

        # ---- PE DVFS warmup: dummy matmuls during the DMA window keep the
        # tensor engine busy from t=0 so real matmuls run at full clock.
        pvw = pvps.tile([128, 2, 512], F32, tag="pvps", name="warmup")
        for _ in range(N_WARMUP):
            nc.tensor.matmul(
                pvw[:, 0, 0 : 2 * (D + 1)], wl[:], wr[:], start=True, stop=True
            )

        # ---- input DMAs, split across both HWDGE dispatch engines (Sync +
        # Act) so the t^T stream keeps up with chunk 0's consumption (the
        # load phase runs at the HBM roofline). 512-col slices only: 1024-col
        # dispatches stall the dispatching engine for 2-5us.
        def tq(a, b):  # t^T column range
            return (t_sb[:, a:b], Ttd[:, a:b])

        # First ~3MB of t^T is the critical stream (chunk 0 consumes ~270GB/s
        # from step 0): fine 256-col shards alternating across both HWDGE
        # dispatch engines to engage the most parallel queues early.
        nc.sync.dma_start(tb_sb[:, 0:256], Tbtd[:, 0:256])
        nc.scalar.dma_start(*tq(0, 256))
        nc.sync.dma_start(*tq(256, 512))
        nc.scalar.dma_start(*tq(512, 768))
        nc.sync.dma_start(*tq(768, 1024))
        nc.scalar.dma_start(zaug[:, 0:4, :], Zar[:, 0:4, :])
        nc.sync.dma_start(*tq(1024, 1280))
        nc.scalar.dma_start(*tq(1280, 1536))
        nc.sync.dma_start(*tq(1536, 1792))
        nc.scalar.dma_start(*tq(1792, 2048))
        nc.sync.dma_start(*tq(2048, 2304))
        nc.scalar.dma_start(zaug[:, 4:8, :], Zar[:, 4:8, :])
        nc.sync.dma_start(*tq(2304, 2560))
        nc.scalar.dma_start(*tq(2560, 2816))
        nc.sync.dma_start(*tq(2816, 3072))
        nc.sync.dma_start(tb_sb[:, 256:1024], Tbtd[:, 256:1024])
        nc.sync.dma_start(*tq(3072, 3584))
        nc.sync.dma_start(zaug[:, 8:16, :], Zar[:, 8:16, :])
        nc.sync.dma_start(*tq(3584, 4096))
        nc.sync.dma_start(*tq(4096, 4608))
        nc.sync.dma_start(zaug[:, 16:32, :], Zar[:, 16:32, :])
        nc.sync.dma_start(*tq(4608, 5120))
        nc.sync.dma_start(zaug[:, 32:48, :], Zar[:, 32:48, :])
        nc.sync.dma_start(zaug[:, 48:64, :], Zar[:, 48:64, :])
        for s in (10, 11, 12, 13, 14, 15):
            nc.sync.dma_start(*tq(512 * s, 512 * (s + 1)))
        nc.sync.dma_start(zbn[:, 0:4, :], Zbr[:, 0:4, :])
        nc.sync.dma_start(zbn[:, 4:8, :], Zbr[:, 4:8, :])

        # ---- main pipeline
        e_tiles = [None] * NCH
        pvt = [None] * NCH

        def emit_norm(c):
            # Za's appended column is 1/TAU, so pv[..., D] = den/TAU and the
            # reciprocal directly yields TAU/den.
            sl = slice(2 * c, 2 * c + 2)
            nc.vector.reciprocal(rec[:, sl, :], pvt[c][:, :, D : D + 1])
            for s01, sg in ((0, 2 * c), (1, 2 * c + 1)):
                nc.scalar.activation(
                    u_sb[:, sg, :],
                    pvt[c][:, s01, 0:D],
                    mybir.ActivationFunctionType.Copy,
                    scale=rec[:, sg, :],
                )  # u = pv * (TAU/den), PSUM -> SBUF
                nc.vector.scalar_tensor_tensor(
                    o_sb[:, sg, :],
                    zbn[:, sg, :],
                    1.0 - TAU,
                    u_sb[:, sg, :],
                    mybir.AluOpType.mult,
                    mybir.AluOpType.add,
                )
            nc.scalar.dma_start(Or[:, sl, :], o_sb[:, sl, :])

        for k in range(NCH * NG + LAG):
            if k < NCH * NG:
                c, g = divmod(k, NG)
                if g == 0:
                    e_tiles[c] = ebig.tile(
                        [128, NT, CH], BF16, tag="ebig", name=f"e_{c}"
                    )
                ps = simps.tile([128, GJ, CH], F32, tag="simps")
                for i in range(GJ):
                    jt = GJ * g + i
                    nc.tensor.matmul(
                        ps[:, i, :],
                        t_sb[:, 128 * jt : 128 * (jt + 1)],
                        tb_sb[:, CH * c : CH * (c + 1)],
                        start=True,
                        stop=True,
                    )
                js = slice(GJ * g, GJ * (g + 1))
                if g in DVE_GROUPS:
                    nc.vector.tensor_scalar(
                        e_tiles[c][:, js, :].bitcast(U16),
                        ps[:, :, :],
                        S_SCHR,
                        b_schr,
                        mybir.AluOpType.mult,
                        mybir.AluOpType.add,
                    )
                else:
                    nc.scalar.activation(
                        e_tiles[c][:, js, :],
                        ps[:, :, :],
                        mybir.ActivationFunctionType.Exp,
                        bias=cbias[:],
                    )
            kp = k - LAG
            if kp >= 0:
                cp, gp = divmod(kp, NG)
                if gp == 0:
                    pvt[cp] = pvps.tile([128, 2, 512], F32, tag="pvps", name=f"pv_{cp}")
                for s01 in (0, 1):
                    for i in range(GJ):
                        jt = GJ * gp + i
                        nc.tensor.matmul(
                            pvt[cp][:, s01, 0 : D + 1],
                            e_tiles[cp][:, jt, 128 * s01 : 128 * (s01 + 1)],
                            zaug[:, jt, :],
                            start=(jt == 0),
                            stop=(jt == NT - 1),
                        )
                if gp == NG - 1:
                    emit_norm(cp)

    return _patch_nc(nc)


# ---------------------------------------------------------------------------

_CACHE = {}


def _get_nc(c_shift: float):
    key = round(float(c_shift), 3)
    if key not in _CACHE:
        _CACHE[key] = _build_nc(key)
    return _CACHE[key]


def prepare(Z, W1, b1, W2, b2):
    """Host-side prep: transform t, softmax shift C, per-core input maps."""
    Z = np.ascontiguousarray(np.asarray(Z, dtype=np.float32))
    W1 = np.ascontiguousarray(np.asarray(W1, dtype=np.float32))
    W2 = np.ascontiguousarray(np.asarray(W2, dtype=np.float32))
    b1 = np.asarray(b1, dtype=np.float32).reshape(1, D)
    b2 = np.asarray(b2, dtype=np.float32).reshape(1, D)

    t = (np.maximum(Z @ W1 + b1, 0.0) @ W2 + b2).astype(np.float32)
    Tt = np.ascontiguousarray(t.T.astype(BF))
    # appended column = 1/TAU so the PV ones-column accumulates den/TAU
    Za = np.concatenate([Z, np.full((N, 1), 1.0 / TAU, np.float32)], axis=1).astype(BF)

    # constant softmax shift C: sim <= max||t||^2 (Cauchy-Schwarz), row
    # maxima >= diag = ||t_i||^2, so this window keeps exp inside fp32
    # range (and uint16 Schraudolph range) and denominators in normal range.
    d2 = np.einsum("nd,nd->n", t, t)
    c_shift = float(min(max(d2.max() - 85.0, 0.0), d2.min() + 80.0))

    in_maps = []
    for c in range(NCORES):
        blk = slice(c * BLK, (c + 1) * BLK)
        in_maps.append(
            {
                "Tt": Tt,
                "Tbt": np.ascontiguousarray(Tt[:, blk]),
                "Za": Za,
                "Zb": Z[blk],
            }
        )
    return in_maps, c_shift


def kernel(Z, W1, b1, W2, b2):
    in_maps, c_shift = prepare(Z, W1, b1, W2, b2)
    nc = _get_nc(c_shift)
    res = run_bass_kernel_spmd(nc, in_maps, list(range(NCORES)))
    return np.concatenate([res.results[c]["O"] for c in range(NCORES)], axis=0)
